# revision 74
# baseline (speedup 1.0000x reference)
"""Multi-head causal attention (B=8, S=2048, E=512, H=8, D=64) on 8 trn2 cores.

Strategy: pure data parallelism over the batch dimension (B == n_cores == 8).
Each NeuronCore computes the full attention for one batch element; no
collectives are needed.

v2 (vs baseline): attention matmul operands in bf16 (1 cyc/row on PE at any
width; fp32r pays 4x under 256 cols), x loaded in 4 batched DMAs instead of
16 (DMA-issue on the sync queue costs ~626ns each), qkT emission interleaved
with stripe-0 attention so ScalarE's exp stream starts early and the PE never
idles long enough for the HAM clock gate to re-throttle to 1.2 GHz, diagonal
causal mask applied as a DVE multiply with a precomputed 0/1 bf16 mask
(instead of a PE mask-matmul), softmax normalization via
reciprocal_approx_fast + PE ones-broadcast (instead of full-precision DVE
reciprocal at ~6.5us and a DRAM round-trip), PSUM->SBUF staging copy and
proj bias-add on GpSimd (idle otherwise), and the exp activation table
pre-warmed at kernel start.

Per core:
  1. Load x [S,E] (4 batched DMAs), transpose on PE -> xT bf16 [E,S].
  2. QKV: qT/kT per 2-head group g from wq/wk (bf16), v stored interleaved
     as vt[p=sk, h, j, 0:64] bf16 with vt[..., 64] = 1.0 so the AV matmul's
     stationary [128, 65] also produces the softmax denominator (row 64).
  3. Per sq-stripe (1024) and head: scoresT[sk,sq] = kT_j^T @ qT (K=64,
     bf16), exp via ScalarE (scale=1/8, no max subtraction needed:
     |scores/8| <~ 2) -> sb bf16, diag block masked by multiplying with a
     strictly-lower-zero mask on DVE, outT_aug[65,sq] += vt_j^T @ attT_j.
     Software-pipelined two deep so the PE does not stall on exp.
  4. Normalize: pst = po staged to SBUF (GpSimd, frees the PSUM bank),
     r = 1/l via reciprocal_approx_fast (DVE), broadcast r across 64
     partitions with a PE ones-matmul, attoutT = pst * r_bcast (DVE, bf16).
  5. Output projection out[s,e] = attoutT^T @ Wp (bf16) + bp (GpSimd add)
     -> HBM, interleaved with the next stripe's attention.

A single PSUM pool with two tags (4+4 banks) is used for the whole kernel.
Post-scheduling, excess semaphore waits are hoisted onto same-engine NoOps
(several ISA structs accept only one wait slot; walrus rejects multi-wait
instructions).
"""

import numpy as np
from contextlib import ExitStack

import concourse.bass as bass
import concourse.mybir as mybir
from concourse.tile import TileContext
from concourse.masks import make_identity
from concourse import bass_utils

F32 = mybir.dt.float32
F32R = mybir.dt.float32r
BF16 = mybir.dt.bfloat16
B, S_FULL, E, H, D = 8, 2048, 512, 8, 64
P = 128
G = H // 2      # 2-head groups
EB = E // P     # e blocks
EXP = mybir.ActivationFunctionType.Exp


def build_attention_nc(S=S_FULL):
    SB = S // P                 # s blocks
    SW = min(1024, S)           # stripe width (sq columns)
    NS = S // SW                # number of stripes
    nc = bass.Bass(trn_type="TRN2")

    x_d = nc.dram_tensor("x", [S, E], F32, kind="ExternalInput").ap()
    wq_d = nc.dram_tensor("Wq", [H, E, D], F32, kind="ExternalInput").ap()
    wk_d = nc.dram_tensor("Wk", [H, E, D], F32, kind="ExternalInput").ap()
    wv_d = nc.dram_tensor("Wv", [H, E, D], F32, kind="ExternalInput").ap()
    wp_d = nc.dram_tensor("Wp", [E, E], F32, kind="ExternalInput").ap()
    bp_d = nc.dram_tensor("bp", [E], F32, kind="ExternalInput").ap()
    out_d = nc.dram_tensor("out", [S, E], F32, kind="ExternalOutput").ap()
    # scratch for the batched softmax-denominator reciprocal: l rows go out
    # in row layout, come back partition-spread [128, H*SW/128], and return
    # reciprocal'd in row layout for the PE broadcast matmul.
    lscr = nc.dram_tensor("lscr", [NS * H * SW], BF16, kind="Internal").ap()
    rscr = nc.dram_tensor("rscr", [NS * H * SW], BF16, kind="Internal").ap()

    with TileContext(nc) as tc, ExitStack() as top:
        const = top.enter_context(tc.tile_pool(name="const", bufs=1))
        warm = const.tile([1, 2], F32, tag="warm")
        nc.vector.memset(warm, 0.0)
        # pre-warm the exp table set (~2.7us) off the critical path
        nc.scalar.activation(out=warm[0:1, 1:2], in_=warm[0:1, 0:1], func=EXP)
        ident = const.tile([P, P], F32, tag="ident")
        make_identity(nc, ident)
        ones = const.tile([1, D], BF16, tag="ones")
        nc.vector.memset(ones, 1.0)
        bpb = const.tile([P, E], F32, tag="bpb")
        wp_sb = const.tile([P, G, E], BF16, tag="wp")

        out_pool = top.enter_context(tc.tile_pool(name="outsb", bufs=2))
        qkv = top.enter_context(tc.tile_pool(name="qkv", bufs=1))
        qT = [qkv.tile([P, S], BF16, tag=f"qT{g}", name=f"qT{g}") for g in range(G)]
        kT = [qkv.tile([P, S], BF16, tag=f"kT{g}", name=f"kT{g}") for g in range(G)]
        vt = qkv.tile([P, H, SB, 65], BF16, tag="vt")
        attp = top.enter_context(tc.tile_pool(name="attsb", bufs=5))

        # single PSUM pool: tag "pa" = 2-bank working tiles x3 (scores can
        # run ahead of exp), tag "po" = attention output accumulator
        # (2 banks x1; freed by the post-AV copies before the next head's
        # first AV needs it) -> 8 banks total
        ppool = top.enter_context(tc.tile_pool(name="ppool", bufs=3, space="PSUM"))

        attout = top.enter_context(tc.tile_pool(name="attout", bufs=1))
        attoutT = [attout.tile([P, G, SW], BF16, tag=f"attoutT{t}",
                               name=f"attoutT{t}") for t in range(NS)]
        rpool = top.enter_context(tc.tile_pool(name="rp", bufs=2))
        # r rows are DMA'd back into the same strip the l rows left from
        # (WAR tracked by the tile layer; saves 32KB/partition of SBUF)
        lrow = [attout.tile([1, H * SW], BF16, tag=f"lrow{t}", name=f"lrow{t}")
                for t in range(NS)]
        rrow = lrow

        # ---------------- attention (per stripe x head) + interleaved proj
        def emit_av(po, h, pend, lo, hi):
            # off=None: sb holds the whole stripe extent (col = c - lo);
            # else: sb is packed at `off` relative to jlo (col = off+c-jlo)
            sb, j, off = pend
            jlo = max(lo, j * P)
            for b in range(lo, hi, 512):
                clo, chi = max(jlo, b), b + 512
                if clo >= chi:
                    continue
                sc = (clo - lo) if off is None else (off + clo - jlo)
                nc.tensor.matmul(po[:, clo - lo:chi - lo],
                                 lhsT=vt[:, h, j, :],
                                 rhs=sb[:, sc:sc + chi - clo],
                                 start=(j == 0), stop=(j == chi // P - 1))

        proj_pp = {}

        def emit_proj(si, glo=0, ghi=G):
            # partial emission (ghi < G) leaves the PSUM accumulator open so
            # the first groups can run before the last heads are normalized
            tt, col = si * P // SW, (si * P) % SW
            if glo == 0:
                proj_pp[si] = ppool.tile([P, E], F32, tag="pa", name="pp")
            pp = proj_pp[si]
            for g in range(glo, ghi):
                nc.tensor.matmul(pp, lhsT=attoutT[tt][:, g, col:col + P],
                                 rhs=wp_sb[:, g, :], start=(g == 0),
                                 stop=(g == G - 1))
            if ghi == G:
                del proj_pp[si]
                ob = out_pool.tile([P, E], F32, tag="ob", name="ob")
                nc.vector.tensor_add(out=ob, in0=pp, in1=bpb)
                # alternate issue queues: ScalarE idles once exp is done,
                # and the 8 trailing output DMAs would serialize on Sync
                eng = nc.sync if si % 2 == 0 else nc.scalar
                eng.dma_start(out=out_d[si * P:(si + 1) * P, :], in_=ob)

        def emit_att_head(t, h, filler=()):
            # `filler` is a list of zero-arg emitters of ~1us of PE-side
            # work, injected between j-blocks: both stripes are ScalarE
            # (exp)-bound, and filler INSIDE the j-pipeline (not between
            # heads, where it would delay the next exp) keeps the PE busy
            # so the HAM clock gate stays at 8/8.
            lo, hi = t * SW, (t + 1) * SW
            jmax = hi // P
            g, hh = h // 2, (h % 2) * D
            filler = list(filler)
            po = ppool.tile([65, SW], F32, tag="po", name="po", bufs=1)
            # group consecutive narrow j-blocks (width <= 512 each) into one
            # PSUM tile / one exp call: the 352-cycle per-ACTIVATE overhead
            # is what makes the causal tail ScalarE-bound
            groups, j = [], 0
            while j < jmax:
                w = hi - max(lo, j * P)
                if w <= 512 and j + 1 < jmax:
                    groups.append([(j, 0), (j + 1, 512)])
                    j += 2
                elif w <= 512:
                    groups.append([(j, 0)])
                    j += 1
                else:
                    groups.append([(j, None)])
                    j += 1
            # stripe-1 heads (many groups) stall in their causal tail,
            # where exp's 352-cycle overhead makes ScalarE lag the PE:
            # concentrate filler there. Short (stripe-0) heads keep the
            # even spread.
            if len(groups) >= 10:
                start = max(1, len(groups) - 2 * len(filler))
                fills = set(range(start, len(groups), 2))
            else:
                fe = (max(2, len(groups) // (len(filler) + 1))
                      if filler else len(groups) + 1)
                fills = set(q for q in range(len(groups)) if q % fe == fe - 1)
            pending = []
            for gi, grp in enumerate(groups):
                if filler and gi in fills:
                    filler.pop(0)()
                ps = ppool.tile([P, SW], F32, tag="pa", name="ps")
                for j, off in grp:
                    jlo = max(lo, j * P)
                    if off is None:
                        for b in range(lo, hi, 512):
                            clo, chi = max(jlo, b), b + 512
                            if clo >= chi:
                                continue
                            nc.tensor.matmul(
                                ps[:, clo - lo:chi - lo],
                                lhsT=kT[g][hh:hh + D, j * P:(j + 1) * P],
                                rhs=qT[g][hh:hh + D, clo:chi],
                                start=True, stop=True)
                    else:
                        nc.tensor.matmul(
                            ps[:, off:off + hi - jlo],
                            lhsT=kT[g][hh:hh + D, j * P:(j + 1) * P],
                            rhs=qT[g][hh:hh + D, jlo:hi],
                            start=True, stop=True)
                while len(pending) >= 4:
                    emit_av(po, h, pending.pop(0), lo, hi)
                sb = attp.tile([P, SW], BF16, tag="attsb", name="sb")
                j0, off0 = grp[0]
                jlo0 = max(lo, j0 * P)
                if off0 is None:
                    ein = (jlo0 - lo, SW)
                else:
                    jl, ol = grp[-1]
                    ein = (0, ol + hi - max(lo, jl * P))
                nc.scalar.activation(
                    out=sb[:, ein[0]:ein[1]], in_=ps[:, ein[0]:ein[1]],
                    func=EXP, scale=float(1.0 / np.sqrt(D)))
                for j, off in grp:
                    if j * P >= lo:
                        # zero the strictly-lower triangle of the diagonal
                        # block (on GpSimd: off the DVE FIFO, which the AV
                        # matmuls depend on via the norm-chain ops)
                        dcol = (j * P - lo) if off is None else off
                        nc.gpsimd.affine_select(
                            out=sb[:, dcol:dcol + P], in_=sb[:, dcol:dcol + P],
                            compare_op=mybir.AluOpType.is_ge, fill=0.0,
                            base=0, pattern=[[1, P]], channel_multiplier=-1)
                    pending.append((sb, j, off))
            while pending:
                emit_av(po, h, pending.pop(0), lo, hi)
            for f in filler:
                f()
            # stage the denominator row l first (it gates the batched
            # reciprocal chain, which is on the tail critical path), then
            # store the UNNORMALIZED attention output (scaled in place once
            # the reciprocal row returns).
            nc.vector.tensor_copy(out=lrow[t][0:1, h * SW:(h + 1) * SW],
                                  in_=po[D:D + 1, :])
            nc.vector.tensor_copy(out=attoutT[t][hh:hh + D, g, :],
                                  in_=po[0:D, :])

        def emit_stripe_recip(t, h0, h1):
            # denominator rows of heads [h0,h1) of stripe t -> DRAM ->
            # partition-spread [128, n/128] -> one cheap DVE reciprocal ->
            # DRAM -> row layout for the per-head PE broadcast.
            n = (h1 - h0) * SW
            off = t * H * SW + h0 * SW
            nc.sync.dma_start(
                out=bass.AP(tensor=lscr.tensor, offset=lscr.offset + off,
                            ap=[[0, 1], [1, n]]),
                in_=lrow[t][0:1, h0 * SW:h1 * SW])
            lsp = rpool.tile([P, n // P], BF16, tag="lsp", name="lsp")
            nc.sync.dma_start(
                out=lsp,
                in_=bass.AP(tensor=lscr.tensor, offset=lscr.offset + off,
                            ap=[[n // P, P], [1, n // P]]))
            rsp = rpool.tile([P, n // P], BF16, tag="rsp", name="rsp")
            with nc.allow_low_precision("softmax denom reciprocal; rel-err "
                                        "budget 2e-2 >> bf16 eps"):
                nc.vector.reciprocal(out=rsp, in_=lsp)
            nc.sync.dma_start(
                out=bass.AP(tensor=rscr.tensor, offset=rscr.offset + off,
                            ap=[[n // P, P], [1, n // P]]),
                in_=rsp)
            nc.sync.dma_start(
                out=rrow[t][0:1, h0 * SW:h1 * SW],
                in_=bass.AP(tensor=rscr.tensor, offset=rscr.offset + off,
                            ap=[[0, 1], [1, n]]))

        def emit_norm(t, h):
            # attoutT[t] *= bcast(1/l) in place
            g, hh = h // 2, (h % 2) * D
            pbc = ppool.tile([D, SW], F32, tag="pa", name="pbc")
            for c in range(0, SW, 512):
                nc.tensor.matmul(
                    pbc[:, c:c + 512], lhsT=ones,
                    rhs=rrow[t][0:1, h * SW + c:h * SW + c + 512],
                    start=True, stop=True)
            sl = attoutT[t][hh:hh + D, g, :]
            nc.vector.tensor_mul(out=sl, in0=sl, in1=pbc)

        # xT and the bf16 weights persist through the attention phase: the
        # stripe-1-only halves of qT/kT and v(8..15) are computed
        # interleaved with the stripe-1 heads as PE filler (stripe-1 is
        # ScalarE-bound; without filler the HAM clock gate re-throttles
        # the PE to 1.2 GHz).
        # xT and the QKV weights are f32r: the weight DMAs write raw fp32
        # bits straight into the matmul operands (no DVE cast on the
        # critical path - the wk cast was gating the first exp), and the
        # xT copies round to f32r. f32r streams 1 cyc/row at >=256 cols,
        # same as bf16 here.
        persist = top.enter_context(tc.tile_pool(name="persist", bufs=1))
        xT = [persist.tile([P, S], F32R, tag=f"xT{e}", name=f"xT{e}")
              for e in range(EB)]
        wq_sb = persist.tile([P, EB, H, D], F32R, tag="wq")
        wk_sb = persist.tile([P, EB, H, D], F32R, tag="wk")
        wv_sb = persist.tile([P, EB, H, D], F32R, tag="wv")

        def emit_v(si):
            pv = ppool.tile([P, E], F32, tag="pa", name="pv")
            for ej in range(EB):
                nc.tensor.matmul(pv, lhsT=xT[ej][:, si * P:(si + 1) * P],
                                 rhs=wv_sb[:, ej], start=(ej == 0),
                                 stop=(ej == EB - 1))
            nc.vector.tensor_copy(out=vt[:, :, si, 0:64],
                                  in_=pv.rearrange("p (h d) -> p h d", h=H))

        def emit_qkt(g, q0, which="kq"):
            pairs = {"k": ((wk_sb, kT[g]),), "q": ((wq_sb, qT[g]),),
                     "kq": ((wk_sb, kT[g]), (wq_sb, qT[g]))}[which]
            for w_sb, dst in pairs:
                pq = ppool.tile([P, 1024], F32, tag="pa", name="pq")
                for ej in range(EB):
                    for c in range(q0, q0 + 1024, 512):
                        nc.tensor.matmul(
                            pq[:, c - q0:c - q0 + 512],
                            lhsT=w_sb[:, ej, 2 * g:2 * g + 2, :],
                            rhs=xT[ej][:, c:c + 512],
                            start=(ej == 0), stop=(ej == EB - 1))
                nc.vector.tensor_copy(out=dst[:, q0:q0 + 1024], in_=pq)

        # filler closures (ordering constraints documented at the schedule)
        def qk(g, q0, w):
            return lambda: emit_qkt(g, q0, w)

        def vf(si):
            return lambda: emit_v(si)

        def nf(t, h):
            return lambda: emit_norm(t, h)

        def pf(si):
            return lambda: emit_proj(si)

        with ExitStack() as ph2:
            xin = ph2.enter_context(tc.tile_pool(name="xin", bufs=4))
            wpool = ph2.enter_context(tc.tile_pool(name="wqkv", bufs=1))

            # ---------------- batched input DMAs. x in 8 half-MB chunks
            # alternating the Sync/Scalar queues (xin bufs=4 so no chunk
            # waits on a staging buffer); wk/wq ride the same two queues
            # right behind x (qkT group 0 gates the first exp); wv/wp on
            # the GpSimd queue. Weight DMAs write raw fp32 bits into the
            # f32r operand tiles directly - no cast, no staging.
            wq_r = wq_d.rearrange("h e d -> e h d").bitcast(F32R)
            wk_r = wk_d.rearrange("h e d -> e h d").bitcast(F32R)
            wv_r = wv_d.rearrange("h e d -> e h d").bitcast(F32R)
            xs4 = [xin.tile([P, 4, E], F32, tag="xs", name=f"xs{k4}")
                   for k4 in range(SB // 4)]
            for k8 in range(SB // 2):
                k4, i2 = k8 // 2, (k8 % 2) * 2
                eng = nc.sync if k8 % 2 == 0 else nc.scalar
                eng.dma_start(
                    out=xs4[k4][:, i2:i2 + 2, :],
                    in_=x_d[k8 * 256:(k8 + 1) * 256, :].rearrange(
                        "(i p) e -> p i e", p=P))
            for ej in range(EB):
                nc.sync.dma_start(out=wk_sb[:, ej],
                                  in_=wk_r[ej * P:(ej + 1) * P, :, :])
                nc.scalar.dma_start(out=wq_sb[:, ej],
                                    in_=wq_r[ej * P:(ej + 1) * P, :, :])
                nc.gpsimd.dma_start(out=wv_sb[:, ej],
                                    in_=wv_r[ej * P:(ej + 1) * P, :, :])
            for g in range(G):
                # GpSimd DMAs can cast: wp goes straight to bf16
                nc.gpsimd.dma_start(out=wp_sb[:, g, :],
                                    in_=wp_d[g * P:(g + 1) * P, :])
            nc.gpsimd.dma_start(
                out=bpb,
                in_=bass.AP(tensor=bp_d.tensor, offset=bp_d.offset,
                            ap=[[0, P]] + list(bp_d.ap)))

            # ---------------- x PE-transpose -> xT bf16 (copies on DVE:
            # ScalarE's strict FIFO would queue stripe-0 exps behind them)
            def emit_tr(k4, ej):
                pt = ppool.tile([P, 512], F32, tag="pa", name="pt")
                for i in range(4):
                    nc.tensor.transpose(pt[:, i * P:(i + 1) * P],
                                        xs4[k4][:, i, ej * P:(ej + 1) * P],
                                        ident)
                nc.vector.tensor_copy(
                    out=xT[ej][:, k4 * 512:(k4 + 1) * 512], in_=pt)

            def trf(k4, ej):
                return lambda: emit_tr(k4, ej)

            # only the first half of x is transposed up front: attention
            # head (0,0) - and with it the whole ScalarE exp rail - starts
            # as soon as x chunks 0-3, wq and wk are in. The second half's
            # transposes ride as filler inside the first two heads.
            for k4 in range(2):
                for ej in range(EB):
                    emit_tr(k4, ej)

            # ---------------- v for stripe-0's key blocks; the rest is
            # emitted as attention filler
            nc.vector.memset(vt[:, :, :, 64:65], 1.0)
            for si in range(SW // P):
                emit_v(si)
            emit_qkt(0, 0)

            emit_att_head(0, 0, [trf(2, 0), trf(2, 1), trf(2, 2), trf(2, 3),
                                 qk(1, 0, "k"), qk(1, 0, "q")])
            emit_att_head(0, 1, [trf(3, 0), trf(3, 1), trf(3, 2), trf(3, 3),
                                 qk(0, SW, "k"), qk(0, SW, "q")])

        # stripe-1 heads are sandwiched between stripe-0 head pairs: a
        # stripe-0 head is ScalarE-light (its exp backlog is short), so the
        # pair's PE surplus covers the sandwiched stripe-1 head's deficit
        # locally and the HAM clock gate never sees a long idle.
        for si in range(SW // P, SW // P + 4):
            emit_v(si)
        emit_att_head(1, 0, [vf(12), vf(13), vf(14), vf(15)])
        emit_att_head(0, 2, [qk(2, 0, "k")])
        emit_att_head(0, 3, [qk(2, 0, "q"), qk(1, SW, "k")])
        emit_att_head(1, 1, [qk(3, 0, "k"), qk(3, 0, "q")])
        emit_att_head(0, 4, [qk(1, SW, "q"), qk(2, SW, "k")])
        emit_att_head(0, 5, [qk(2, SW, "q")])
        emit_att_head(1, 2, [qk(3, SW, "k"), qk(3, SW, "q")])
        emit_att_head(0, 6)
        emit_att_head(0, 7)
        emit_stripe_recip(0, 0, H)
        emit_att_head(1, 3)
        emit_stripe_recip(1, 0, 4)
        emit_att_head(1, 4, [nf(0, 0), nf(0, 1), nf(0, 2), nf(0, 3)])
        emit_att_head(1, 5, [nf(0, 4), nf(0, 5), nf(0, 6), nf(0, 7)])
        emit_att_head(1, 6, [pf(0), pf(1), pf(2), nf(1, 0), nf(1, 1)])
        emit_stripe_recip(1, 4, 7)
        emit_att_head(1, 7, [pf(3), pf(4), pf(5), nf(1, 2), nf(1, 3)])
        emit_proj(6)
        emit_proj(7)
        # only head 7's own (tiny [128,8]-spread) reciprocal chain remains
        # serial; norms 4-6 and the partial projs overlap it
        emit_stripe_recip(1, 7, H)
        for h in range(4, 7):
            emit_norm(1, h)
        # groups 0-2 of the first trailing projs only need heads 0-5: they
        # fill the PE while head 7's reciprocal/norm completes (only 2
        # partials: they hold pa buffers, and norm(1,7)'s broadcast needs
        # the third)
        for si in range(SW // P, SW // P + 2):
            emit_proj(si, 0, 3)
        emit_norm(1, 7)
        for si in range(SW // P, SW // P + 2):
            emit_proj(si, 3, G)
        for si in range(SW // P + 2, S // P):
            emit_proj(si)

    _hoist_matmul_waits(nc)
    return nc


def _hoist_matmul_waits(nc):
    """Several TRN2 ISA structs (notably the fp32 self-loading Matmult's LW
    struct) accept only one sync-wait slot; walrus dies with "Too many sync
    wait commands" otherwise. Hoist every wait of a multi-wait instruction
    onto same-engine NoOps inserted right before it (same engine queue =>
    identical ordering semantics)."""
    nid = [0]
    for fn in nc.m.functions:
        for blk in fn.blocks:
            insts = blk.instructions
            out = []
            for inst in insts:
                si = inst.sync_info
                if (inst.engine != mybir.EngineType.Unassigned and si is not None
                        and len(si.on_wait) >= 2 and inst.opcode != "NoOp"):
                    for w in si.on_wait:
                        nid[0] += 1
                        nop = mybir.InstNoOp(name=f"I-mmwait-{nid[0]}",
                                             ins=[], outs=[])
                        nop.engine = inst.engine
                        nop.sync_info = mybir.SyncInfo(on_wait=[w], on_update=[])
                        nc.inst_map[nop.name] = nop
                        out.append(nop)
                    inst.sync_info = mybir.SyncInfo(on_wait=[],
                                                    on_update=list(si.on_update))
                out.append(inst)
            if len(out) != len(insts):
                insts[:] = out


_nc_cache = {}


def _get_nc(S=S_FULL):
    if S not in _nc_cache:
        _nc_cache[S] = build_attention_nc(S)
    return _nc_cache[S]


def kernel(x, Wq, Wk, Wv, Wp, bp, _trace=False):
    nc = _get_nc(x.shape[1])
    n = x.shape[0]
    wq = np.ascontiguousarray(Wq, np.float32)
    wk = np.ascontiguousarray(Wk, np.float32)
    wv = np.ascontiguousarray(Wv, np.float32)
    wp = np.ascontiguousarray(Wp, np.float32)
    bpc = np.ascontiguousarray(bp, np.float32)
    in_maps = [
        {"x": np.ascontiguousarray(x[b], np.float32),
         "Wq": wq, "Wk": wk, "Wv": wv, "Wp": wp, "bp": bpc}
        for b in range(n)
    ]
    res = bass_utils.run_bass_kernel_spmd(
        nc, in_maps, core_ids=list(range(n)), trace=_trace)
    out = np.stack([r["out"] for r in res.results], axis=0)
    if _trace:
        return out, res
    return out


# revision 75
# speedup vs baseline: 1.0183x; 1.0183x over previous
"""Multi-head causal attention (B=8, S=2048, E=512, H=8, D=64) on 8 trn2 cores.

Strategy: pure data parallelism over the batch dimension (B == n_cores == 8).
Each NeuronCore computes the full attention for one batch element; no
collectives are needed.

v2 (vs baseline): attention matmul operands in bf16 (1 cyc/row on PE at any
width; fp32r pays 4x under 256 cols), x loaded in 4 batched DMAs instead of
16 (DMA-issue on the sync queue costs ~626ns each), qkT emission interleaved
with stripe-0 attention so ScalarE's exp stream starts early and the PE never
idles long enough for the HAM clock gate to re-throttle to 1.2 GHz, diagonal
causal mask applied as a DVE multiply with a precomputed 0/1 bf16 mask
(instead of a PE mask-matmul), softmax normalization via
reciprocal_approx_fast + PE ones-broadcast (instead of full-precision DVE
reciprocal at ~6.5us and a DRAM round-trip), PSUM->SBUF staging copy and
proj bias-add on GpSimd (idle otherwise), and the exp activation table
pre-warmed at kernel start.

Per core:
  1. Load x [S,E] (4 batched DMAs), transpose on PE -> xT bf16 [E,S].
  2. QKV: qT/kT per 2-head group g from wq/wk (bf16), v stored interleaved
     as vt[p=sk, h, j, 0:64] bf16 with vt[..., 64] = 1.0 so the AV matmul's
     stationary [128, 65] also produces the softmax denominator (row 64).
  3. Per sq-stripe (1024) and head: scoresT[sk,sq] = kT_j^T @ qT (K=64,
     bf16), exp via ScalarE (scale=1/8, no max subtraction needed:
     |scores/8| <~ 2) -> sb bf16, diag block masked by multiplying with a
     strictly-lower-zero mask on DVE, outT_aug[65,sq] += vt_j^T @ attT_j.
     Software-pipelined two deep so the PE does not stall on exp.
  4. Normalize: pst = po staged to SBUF (GpSimd, frees the PSUM bank),
     r = 1/l via reciprocal_approx_fast (DVE), broadcast r across 64
     partitions with a PE ones-matmul, attoutT = pst * r_bcast (DVE, bf16).
  5. Output projection out[s,e] = attoutT^T @ Wp (bf16) + bp (GpSimd add)
     -> HBM, interleaved with the next stripe's attention.

A single PSUM pool with two tags (4+4 banks) is used for the whole kernel.
Post-scheduling, excess semaphore waits are hoisted onto same-engine NoOps
(several ISA structs accept only one wait slot; walrus rejects multi-wait
instructions).
"""

import numpy as np
from contextlib import ExitStack

import concourse.bass as bass
import concourse.mybir as mybir
from concourse.tile import TileContext
from concourse.masks import make_identity
from concourse import bass_utils

F32 = mybir.dt.float32
F32R = mybir.dt.float32r
BF16 = mybir.dt.bfloat16
B, S_FULL, E, H, D = 8, 2048, 512, 8, 64
P = 128
G = H // 2      # 2-head groups
EB = E // P     # e blocks
EXP = mybir.ActivationFunctionType.Exp


def build_attention_nc(S=S_FULL):
    SB = S // P                 # s blocks
    SW = min(1024, S)           # stripe width (sq columns)
    NS = S // SW                # number of stripes
    nc = bass.Bass(trn_type="TRN2")

    x_d = nc.dram_tensor("x", [S, E], F32, kind="ExternalInput").ap()
    wq_d = nc.dram_tensor("Wq", [H, E, D], F32, kind="ExternalInput").ap()
    wk_d = nc.dram_tensor("Wk", [H, E, D], F32, kind="ExternalInput").ap()
    wv_d = nc.dram_tensor("Wv", [H, E, D], F32, kind="ExternalInput").ap()
    wp_d = nc.dram_tensor("Wp", [E, E], F32, kind="ExternalInput").ap()
    bp_d = nc.dram_tensor("bp", [E], F32, kind="ExternalInput").ap()
    out_d = nc.dram_tensor("out", [S, E], F32, kind="ExternalOutput").ap()
    # scratch for the batched softmax-denominator reciprocal: l rows go out
    # in row layout, come back partition-spread [128, H*SW/128], and return
    # reciprocal'd in row layout for the PE broadcast matmul.
    lscr = nc.dram_tensor("lscr", [NS * H * SW], BF16, kind="Internal").ap()
    rscr = nc.dram_tensor("rscr", [NS * H * SW], BF16, kind="Internal").ap()

    with TileContext(nc) as tc, ExitStack() as top:
        const = top.enter_context(tc.tile_pool(name="const", bufs=1))
        warm = const.tile([1, 2], F32, tag="warm")
        nc.vector.memset(warm, 0.0)
        # pre-warm the exp table set (~2.7us) off the critical path
        nc.scalar.activation(out=warm[0:1, 1:2], in_=warm[0:1, 0:1], func=EXP)
        ident = const.tile([P, P], F32, tag="ident")
        make_identity(nc, ident)
        ones = const.tile([1, D], BF16, tag="ones")
        nc.vector.memset(ones, 1.0)
        bpb = const.tile([P, E], F32, tag="bpb")
        wp_sb = const.tile([P, G, E], BF16, tag="wp")

        out_pool = top.enter_context(tc.tile_pool(name="outsb", bufs=2))
        qkv = top.enter_context(tc.tile_pool(name="qkv", bufs=1))
        qT = [qkv.tile([P, S], BF16, tag=f"qT{g}", name=f"qT{g}") for g in range(G)]
        kT = [qkv.tile([P, S], BF16, tag=f"kT{g}", name=f"kT{g}") for g in range(G)]
        vt = qkv.tile([P, H, SB, 65], BF16, tag="vt")
        attp = top.enter_context(tc.tile_pool(name="attsb", bufs=5))

        # single PSUM pool: tag "pa" = 2-bank working tiles x3 (scores can
        # run ahead of exp), tag "po" = attention output accumulator
        # (2 banks x1; freed by the post-AV copies before the next head's
        # first AV needs it) -> 8 banks total
        ppool = top.enter_context(tc.tile_pool(name="ppool", bufs=3, space="PSUM"))

        attout = top.enter_context(tc.tile_pool(name="attout", bufs=1))
        attoutT = [attout.tile([P, G, SW], BF16, tag=f"attoutT{t}",
                               name=f"attoutT{t}") for t in range(NS)]
        rpool = top.enter_context(tc.tile_pool(name="rp", bufs=2))
        # r rows are DMA'd back into the same strip the l rows left from
        # (WAR tracked by the tile layer; saves 32KB/partition of SBUF)
        lrow = [attout.tile([1, H * SW], BF16, tag=f"lrow{t}", name=f"lrow{t}")
                for t in range(NS)]
        rrow = lrow

        # ---------------- attention (per stripe x head) + interleaved proj
        def emit_av(po, h, pend, lo, hi):
            # off=None: sb holds the whole stripe extent (col = c - lo);
            # else: sb is packed at `off` relative to jlo (col = off+c-jlo)
            sb, j, off = pend
            jlo = max(lo, j * P)
            for b in range(lo, hi, 512):
                clo, chi = max(jlo, b), b + 512
                if clo >= chi:
                    continue
                sc = (clo - lo) if off is None else (off + clo - jlo)
                nc.tensor.matmul(po[:, clo - lo:chi - lo],
                                 lhsT=vt[:, h, j, :],
                                 rhs=sb[:, sc:sc + chi - clo],
                                 start=(j == 0), stop=(j == chi // P - 1))

        proj_pp = {}

        def emit_proj(si, glo=0, ghi=G):
            # partial emission (ghi < G) leaves the PSUM accumulator open so
            # the first groups can run before the last heads are normalized
            tt, col = si * P // SW, (si * P) % SW
            if glo == 0:
                proj_pp[si] = ppool.tile([P, E], F32, tag="pa", name="pp")
            pp = proj_pp[si]
            for g in range(glo, ghi):
                nc.tensor.matmul(pp, lhsT=attoutT[tt][:, g, col:col + P],
                                 rhs=wp_sb[:, g, :], start=(g == 0),
                                 stop=(g == G - 1))
            if ghi == G:
                del proj_pp[si]
                ob = out_pool.tile([P, E], F32, tag="ob", name="ob")
                nc.vector.tensor_add(out=ob, in0=pp, in1=bpb)
                nc.sync.dma_start(out=out_d[si * P:(si + 1) * P, :], in_=ob)

        def emit_att_head(t, h, filler=()):
            # `filler` is a list of zero-arg emitters of ~1us of PE-side
            # work, injected between j-blocks: both stripes are ScalarE
            # (exp)-bound, and filler INSIDE the j-pipeline (not between
            # heads, where it would delay the next exp) keeps the PE busy
            # so the HAM clock gate stays at 8/8.
            lo, hi = t * SW, (t + 1) * SW
            jmax = hi // P
            g, hh = h // 2, (h % 2) * D
            filler = list(filler)
            po = ppool.tile([65, SW], F32, tag="po", name="po", bufs=1)
            # group consecutive narrow j-blocks (width <= 512 each) into one
            # PSUM tile / one exp call: the 352-cycle per-ACTIVATE overhead
            # is what makes the causal tail ScalarE-bound
            groups, j = [], 0
            while j < jmax:
                w = hi - max(lo, j * P)
                if w <= 512 and j + 1 < jmax:
                    groups.append([(j, 0), (j + 1, 512)])
                    j += 2
                elif w <= 512:
                    groups.append([(j, 0)])
                    j += 1
                else:
                    groups.append([(j, None)])
                    j += 1
            # stripe-1 heads (many groups) stall in their causal tail,
            # where exp's 352-cycle overhead makes ScalarE lag the PE:
            # concentrate filler there. Short (stripe-0) heads keep the
            # even spread.
            if len(groups) >= 10:
                start = max(1, len(groups) - 2 * len(filler))
                fills = set(range(start, len(groups), 2))
            else:
                fe = (max(2, len(groups) // (len(filler) + 1))
                      if filler else len(groups) + 1)
                fills = set(q for q in range(len(groups)) if q % fe == fe - 1)
            pending = []
            for gi, grp in enumerate(groups):
                if filler and gi in fills:
                    filler.pop(0)()
                ps = ppool.tile([P, SW], F32, tag="pa", name="ps")
                for j, off in grp:
                    jlo = max(lo, j * P)
                    if off is None:
                        for b in range(lo, hi, 512):
                            clo, chi = max(jlo, b), b + 512
                            if clo >= chi:
                                continue
                            nc.tensor.matmul(
                                ps[:, clo - lo:chi - lo],
                                lhsT=kT[g][hh:hh + D, j * P:(j + 1) * P],
                                rhs=qT[g][hh:hh + D, clo:chi],
                                start=True, stop=True)
                    else:
                        nc.tensor.matmul(
                            ps[:, off:off + hi - jlo],
                            lhsT=kT[g][hh:hh + D, j * P:(j + 1) * P],
                            rhs=qT[g][hh:hh + D, jlo:hi],
                            start=True, stop=True)
                while len(pending) >= 4:
                    emit_av(po, h, pending.pop(0), lo, hi)
                sb = attp.tile([P, SW], BF16, tag="attsb", name="sb")
                j0, off0 = grp[0]
                jlo0 = max(lo, j0 * P)
                if off0 is None:
                    ein = (jlo0 - lo, SW)
                else:
                    jl, ol = grp[-1]
                    ein = (0, ol + hi - max(lo, jl * P))
                nc.scalar.activation(
                    out=sb[:, ein[0]:ein[1]], in_=ps[:, ein[0]:ein[1]],
                    func=EXP, scale=float(1.0 / np.sqrt(D)))
                for j, off in grp:
                    if j * P >= lo:
                        # zero the strictly-lower triangle of the diagonal
                        # block (on GpSimd: off the DVE FIFO, which the AV
                        # matmuls depend on via the norm-chain ops)
                        dcol = (j * P - lo) if off is None else off
                        nc.gpsimd.affine_select(
                            out=sb[:, dcol:dcol + P], in_=sb[:, dcol:dcol + P],
                            compare_op=mybir.AluOpType.is_ge, fill=0.0,
                            base=0, pattern=[[1, P]], channel_multiplier=-1)
                    pending.append((sb, j, off))
            while pending:
                emit_av(po, h, pending.pop(0), lo, hi)
            for f in filler:
                f()
            # stage the denominator row l first (it gates the batched
            # reciprocal chain, which is on the tail critical path), then
            # store the UNNORMALIZED attention output (scaled in place once
            # the reciprocal row returns).
            nc.vector.tensor_copy(out=lrow[t][0:1, h * SW:(h + 1) * SW],
                                  in_=po[D:D + 1, :])
            nc.vector.tensor_copy(out=attoutT[t][hh:hh + D, g, :],
                                  in_=po[0:D, :])

        def emit_stripe_recip(t, h0, h1):
            # denominator rows of heads [h0,h1) of stripe t -> DRAM ->
            # partition-spread [128, n/128] -> one cheap DVE reciprocal ->
            # DRAM -> row layout for the per-head PE broadcast.
            n = (h1 - h0) * SW
            off = t * H * SW + h0 * SW
            nc.sync.dma_start(
                out=bass.AP(tensor=lscr.tensor, offset=lscr.offset + off,
                            ap=[[0, 1], [1, n]]),
                in_=lrow[t][0:1, h0 * SW:h1 * SW])
            lsp = rpool.tile([P, n // P], BF16, tag="lsp", name="lsp")
            nc.sync.dma_start(
                out=lsp,
                in_=bass.AP(tensor=lscr.tensor, offset=lscr.offset + off,
                            ap=[[n // P, P], [1, n // P]]))
            rsp = rpool.tile([P, n // P], BF16, tag="rsp", name="rsp")
            with nc.allow_low_precision("softmax denom reciprocal; rel-err "
                                        "budget 2e-2 >> bf16 eps"):
                nc.vector.reciprocal(out=rsp, in_=lsp)
            nc.sync.dma_start(
                out=bass.AP(tensor=rscr.tensor, offset=rscr.offset + off,
                            ap=[[n // P, P], [1, n // P]]),
                in_=rsp)
            nc.sync.dma_start(
                out=rrow[t][0:1, h0 * SW:h1 * SW],
                in_=bass.AP(tensor=rscr.tensor, offset=rscr.offset + off,
                            ap=[[0, 1], [1, n]]))

        def emit_norm(t, h):
            # attoutT[t] *= bcast(1/l) in place
            g, hh = h // 2, (h % 2) * D
            pbc = ppool.tile([D, SW], F32, tag="pa", name="pbc")
            for c in range(0, SW, 512):
                nc.tensor.matmul(
                    pbc[:, c:c + 512], lhsT=ones,
                    rhs=rrow[t][0:1, h * SW + c:h * SW + c + 512],
                    start=True, stop=True)
            sl = attoutT[t][hh:hh + D, g, :]
            nc.vector.tensor_mul(out=sl, in0=sl, in1=pbc)

        # xT and the bf16 weights persist through the attention phase: the
        # stripe-1-only halves of qT/kT and v(8..15) are computed
        # interleaved with the stripe-1 heads as PE filler (stripe-1 is
        # ScalarE-bound; without filler the HAM clock gate re-throttles
        # the PE to 1.2 GHz).
        # xT and the QKV weights are f32r: the weight DMAs write raw fp32
        # bits straight into the matmul operands (no DVE cast on the
        # critical path - the wk cast was gating the first exp), and the
        # xT copies round to f32r. f32r streams 1 cyc/row at >=256 cols,
        # same as bf16 here.
        persist = top.enter_context(tc.tile_pool(name="persist", bufs=1))
        xT = [persist.tile([P, S], F32R, tag=f"xT{e}", name=f"xT{e}")
              for e in range(EB)]
        wq_sb = persist.tile([P, EB, H, D], F32R, tag="wq")
        wk_sb = persist.tile([P, EB, H, D], F32R, tag="wk")
        wv_sb = persist.tile([P, EB, H, D], F32R, tag="wv")

        def emit_v(si):
            pv = ppool.tile([P, E], F32, tag="pa", name="pv")
            for ej in range(EB):
                nc.tensor.matmul(pv, lhsT=xT[ej][:, si * P:(si + 1) * P],
                                 rhs=wv_sb[:, ej], start=(ej == 0),
                                 stop=(ej == EB - 1))
            nc.vector.tensor_copy(out=vt[:, :, si, 0:64],
                                  in_=pv.rearrange("p (h d) -> p h d", h=H))

        def emit_qkt(g, q0, which="kq"):
            pairs = {"k": ((wk_sb, kT[g]),), "q": ((wq_sb, qT[g]),),
                     "kq": ((wk_sb, kT[g]), (wq_sb, qT[g]))}[which]
            for w_sb, dst in pairs:
                pq = ppool.tile([P, 1024], F32, tag="pa", name="pq")
                for ej in range(EB):
                    for c in range(q0, q0 + 1024, 512):
                        nc.tensor.matmul(
                            pq[:, c - q0:c - q0 + 512],
                            lhsT=w_sb[:, ej, 2 * g:2 * g + 2, :],
                            rhs=xT[ej][:, c:c + 512],
                            start=(ej == 0), stop=(ej == EB - 1))
                nc.vector.tensor_copy(out=dst[:, q0:q0 + 1024], in_=pq)

        # filler closures (ordering constraints documented at the schedule)
        def qk(g, q0, w):
            return lambda: emit_qkt(g, q0, w)

        def vf(si):
            return lambda: emit_v(si)

        def nf(t, h):
            return lambda: emit_norm(t, h)

        def pf(si):
            return lambda: emit_proj(si)

        with ExitStack() as ph2:
            xin = ph2.enter_context(tc.tile_pool(name="xin", bufs=4))
            wpool = ph2.enter_context(tc.tile_pool(name="wqkv", bufs=1))

            # ---------------- batched input DMAs. x in 8 half-MB chunks
            # alternating the Sync/Scalar queues (xin bufs=4 so no chunk
            # waits on a staging buffer); wk/wq ride the same two queues
            # right behind x (qkT group 0 gates the first exp); wv/wp on
            # the GpSimd queue. Weight DMAs write raw fp32 bits into the
            # f32r operand tiles directly - no cast, no staging.
            wq_r = wq_d.rearrange("h e d -> e h d").bitcast(F32R)
            wk_r = wk_d.rearrange("h e d -> e h d").bitcast(F32R)
            wv_r = wv_d.rearrange("h e d -> e h d").bitcast(F32R)
            xs4 = [xin.tile([P, 4, E], F32, tag="xs", name=f"xs{k4}")
                   for k4 in range(SB // 4)]
            for k8 in range(SB // 2):
                k4, i2 = k8 // 2, (k8 % 2) * 2
                eng = nc.sync if k8 % 2 == 0 else nc.scalar
                eng.dma_start(
                    out=xs4[k4][:, i2:i2 + 2, :],
                    in_=x_d[k8 * 256:(k8 + 1) * 256, :].rearrange(
                        "(i p) e -> p i e", p=P))
            for ej in range(EB):
                nc.sync.dma_start(out=wk_sb[:, ej],
                                  in_=wk_r[ej * P:(ej + 1) * P, :, :])
                nc.scalar.dma_start(out=wq_sb[:, ej],
                                    in_=wq_r[ej * P:(ej + 1) * P, :, :])
                nc.gpsimd.dma_start(out=wv_sb[:, ej],
                                    in_=wv_r[ej * P:(ej + 1) * P, :, :])
            for g in range(G):
                # GpSimd DMAs can cast: wp goes straight to bf16
                nc.gpsimd.dma_start(out=wp_sb[:, g, :],
                                    in_=wp_d[g * P:(g + 1) * P, :])
            nc.gpsimd.dma_start(
                out=bpb,
                in_=bass.AP(tensor=bp_d.tensor, offset=bp_d.offset,
                            ap=[[0, P]] + list(bp_d.ap)))

            # ---------------- x PE-transpose -> xT bf16 (copies on DVE:
            # ScalarE's strict FIFO would queue stripe-0 exps behind them)
            def emit_tr(k4, ej):
                pt = ppool.tile([P, 512], F32, tag="pa", name="pt")
                for i in range(4):
                    nc.tensor.transpose(pt[:, i * P:(i + 1) * P],
                                        xs4[k4][:, i, ej * P:(ej + 1) * P],
                                        ident)
                nc.vector.tensor_copy(
                    out=xT[ej][:, k4 * 512:(k4 + 1) * 512], in_=pt)

            def trf(k4, ej):
                return lambda: emit_tr(k4, ej)

            # only the first half of x is transposed up front: attention
            # head (0,0) - and with it the whole ScalarE exp rail - starts
            # as soon as x chunks 0-3, wq and wk are in. The second half's
            # transposes ride as filler inside the first two heads.
            for k4 in range(2):
                for ej in range(EB):
                    emit_tr(k4, ej)

            # ---------------- v for stripe-0's key blocks; the rest is
            # emitted as attention filler
            nc.vector.memset(vt[:, :, :, 64:65], 1.0)
            for si in range(SW // P):
                emit_v(si)
            emit_qkt(0, 0)

            emit_att_head(0, 0, [trf(2, 0), trf(2, 1), trf(2, 2), trf(2, 3),
                                 qk(1, 0, "k"), qk(1, 0, "q")])
            emit_att_head(0, 1, [trf(3, 0), trf(3, 1), trf(3, 2), trf(3, 3),
                                 qk(0, SW, "k"), qk(0, SW, "q")])

        # stripe-1 heads are sandwiched between stripe-0 head pairs: a
        # stripe-0 head is ScalarE-light (its exp backlog is short), so the
        # pair's PE surplus covers the sandwiched stripe-1 head's deficit
        # locally and the HAM clock gate never sees a long idle.
        for si in range(SW // P, SW // P + 4):
            emit_v(si)
        emit_att_head(1, 0, [vf(12), vf(13), vf(14), vf(15)])
        emit_att_head(0, 2, [qk(2, 0, "k")])
        emit_att_head(0, 3, [qk(2, 0, "q"), qk(1, SW, "k")])
        emit_att_head(1, 1, [qk(3, 0, "k"), qk(3, 0, "q")])
        emit_att_head(0, 4, [qk(1, SW, "q"), qk(2, SW, "k")])
        emit_att_head(0, 5, [qk(2, SW, "q")])
        emit_att_head(1, 2, [qk(3, SW, "k"), qk(3, SW, "q")])
        emit_att_head(0, 6)
        emit_att_head(0, 7)
        emit_stripe_recip(0, 0, H)
        emit_att_head(1, 3)
        emit_stripe_recip(1, 0, 4)
        emit_att_head(1, 4, [nf(0, 0), nf(0, 1), nf(0, 2), nf(0, 3)])
        emit_att_head(1, 5, [nf(0, 4), nf(0, 5), nf(0, 6), nf(0, 7)])
        emit_att_head(1, 6, [pf(0), pf(1), pf(2), nf(1, 0), nf(1, 1)])
        emit_stripe_recip(1, 4, 7)
        emit_att_head(1, 7, [pf(3), pf(4), pf(5), nf(1, 2), nf(1, 3)])
        emit_proj(6)
        emit_proj(7)
        # only head 7's own (tiny [128,8]-spread) reciprocal chain remains
        # serial; norms 4-6 and the partial projs overlap it
        emit_stripe_recip(1, 7, H)
        for h in range(4, 7):
            emit_norm(1, h)
        # groups 0-2 of the first trailing projs only need heads 0-5: they
        # fill the PE while head 7's reciprocal/norm completes (only 2
        # partials: they hold pa buffers, and norm(1,7)'s broadcast needs
        # the third)
        for si in range(SW // P, SW // P + 2):
            emit_proj(si, 0, 3)
        emit_norm(1, 7)
        for si in range(SW // P, SW // P + 2):
            emit_proj(si, 3, G)
        for si in range(SW // P + 2, S // P):
            emit_proj(si)

    _hoist_matmul_waits(nc)
    return nc


def _hoist_matmul_waits(nc):
    """Several TRN2 ISA structs (notably the fp32 self-loading Matmult's LW
    struct) accept only one sync-wait slot; walrus dies with "Too many sync
    wait commands" otherwise. Hoist every wait of a multi-wait instruction
    onto same-engine NoOps inserted right before it (same engine queue =>
    identical ordering semantics)."""
    nid = [0]
    for fn in nc.m.functions:
        for blk in fn.blocks:
            insts = blk.instructions
            out = []
            for inst in insts:
                si = inst.sync_info
                if (inst.engine != mybir.EngineType.Unassigned and si is not None
                        and len(si.on_wait) >= 2 and inst.opcode != "NoOp"):
                    for w in si.on_wait:
                        nid[0] += 1
                        nop = mybir.InstNoOp(name=f"I-mmwait-{nid[0]}",
                                             ins=[], outs=[])
                        nop.engine = inst.engine
                        nop.sync_info = mybir.SyncInfo(on_wait=[w], on_update=[])
                        nc.inst_map[nop.name] = nop
                        out.append(nop)
                    inst.sync_info = mybir.SyncInfo(on_wait=[],
                                                    on_update=list(si.on_update))
                out.append(inst)
            if len(out) != len(insts):
                insts[:] = out


_nc_cache = {}


def _get_nc(S=S_FULL):
    if S not in _nc_cache:
        _nc_cache[S] = build_attention_nc(S)
    return _nc_cache[S]


def kernel(x, Wq, Wk, Wv, Wp, bp, _trace=False):
    nc = _get_nc(x.shape[1])
    n = x.shape[0]
    wq = np.ascontiguousarray(Wq, np.float32)
    wk = np.ascontiguousarray(Wk, np.float32)
    wv = np.ascontiguousarray(Wv, np.float32)
    wp = np.ascontiguousarray(Wp, np.float32)
    bpc = np.ascontiguousarray(bp, np.float32)
    in_maps = [
        {"x": np.ascontiguousarray(x[b], np.float32),
         "Wq": wq, "Wk": wk, "Wv": wv, "Wp": wp, "bp": bpc}
        for b in range(n)
    ]
    res = bass_utils.run_bass_kernel_spmd(
        nc, in_maps, core_ids=list(range(n)), trace=_trace)
    out = np.stack([r["out"] for r in res.results], axis=0)
    if _trace:
        return out, res
    return out
